# revision 9
# baseline (speedup 1.0000x reference)
"""GQA sliding-window attention block, tensor-parallel over 8 TRN2 NeuronCores.

Each core owns 4 query heads + their shared KV head (GQA group), computes
qkv-proj for its slice, banded flash-style attention (sliding window 512
=> only 5 key blocks of 128 per 128-query block), and a partial
out-projection (contraction over its 256 attention-output dims).  The host
sums the 8 partial outputs.

Per-core pipeline is striped over the 16 blocks of 128 sequence positions:
  stripe m: qkv matmul -> rmsnorm+rope (fused tables) -> PE transposes into
            qT/kT -> scores for key-block m-4 (needs q up to block m) ->
            exp+mask (probs, bf16) -> PV for query-block m-4 -> out-proj ->
            DMA partial out.
"""

import os
import sys

import numpy as np

if "/opt/trn_rl_repo" not in sys.path:
    sys.path.insert(0, "/opt/trn_rl_repo")

import concourse.bass as bass
from concourse import bacc
import concourse.mybir as mybir
import concourse.tile as tile
from concourse.bass_utils import run_bass_kernel_spmd
from concourse.masks import make_identity

F32 = mybir.dt.float32
F32R = mybir.dt.float32r
BF16 = mybir.dt.bfloat16
AF = mybir.ActivationFunctionType
ALU = mybir.AluOpType

N_HEADS = 32
N_KV_HEADS = 8
HD = 64
D_MODEL = 2048
S = 2048
WINDOW = 512
THETA = 10000.0
EPS = 1e-6
NCORES = 8
HPC = N_HEADS // NCORES          # 4 q heads per core
QO = HPC * HD                    # 256 q cols per core
O = QO + 2 * HD                  # 384 qkv cols per core
SCALE = HD ** -0.5
NS = S // 128                    # 16 stripes
KBW = WINDOW // 128 + 1          # 5 key blocks per query block
WMAX = KBW * 128                 # 640

_NC = None


def _body(tc, xT, wct, wot, cos2, sin2, out):
    nc = tc.nc
    with (
        tc.tile_pool(name="const", bufs=1) as constp,
        tc.tile_pool(name="xk", bufs=2) as xkp,
        tc.tile_pool(name="work", bufs=2) as workp,
        tc.tile_pool(name="small", bufs=4) as smallp,
        tc.tile_pool(name="persist", bufs=1) as persistp,
        tc.tile_pool(name="probs", bufs=24) as probsp,
        tc.tile_pool(name="attn", bufs=2) as attnp,
        tc.tile_pool(name="outs", bufs=2) as outsp,
        tc.tile_pool(name="psA", bufs=2, space="PSUM") as psA,
        tc.tile_pool(name="psSC", bufs=2, space="PSUM") as psSC,
        tc.tile_pool(name="psPV", bufs=1, space="PSUM") as psPV,
        tc.tile_pool(name="psTR", bufs=1, space="PSUM") as psTR,
    ):
        ident = constp.tile([128, 128], F32)
        make_identity(nc, ident)
        eps_sb = constp.tile([128, 1], F32)
        nc.vector.memset(eps_sb, EPS)

        wct_sb = constp.tile([128, 16, O], F32R)
        nc.sync.dma_start(out=wct_sb, in_=wct.rearrange("(kc p) o -> p kc o", p=128))
        wot_sb = constp.tile([128, 2, D_MODEL], F32R)
        nc.sync.dma_start(out=wot_sb, in_=wot.rearrange("(oc p) n -> p oc n", p=128))
        qT = [
            persistp.tile([64, S], F32R, name=f"qT{h}", tag=f"qT{h}")
            for h in range(HPC)
        ]
        kT = persistp.tile([64, S], F32R)
        v_sb = persistp.tile([128, NS, HD + 1], BF16)
        nc.vector.memset(v_sb[:, :, HD:HD + 1], 1.0)

        probs_tiles = {}

        for m in range(NS + 4):
            if m < NS:
                # ---- load x stripe (all d, 128 s-cols), as [p=d%128, kc, s] ----
                xTm = xkp.tile([128, 16, 128], F32R, tag="xTm")
                nc.sync.dma_start(
                    out=xTm,
                    in_=xT[:, m * 128:(m + 1) * 128].rearrange(
                        "(kc p) s -> p kc s", p=128
                    ),
                )
                cos_m = xkp.tile([128, 2 * HD], F32, tag="cosm")
                nc.sync.dma_start(out=cos_m, in_=cos2[m * 128:(m + 1) * 128, :])
                sin_m = xkp.tile([128, 2 * HD], F32, tag="sinm")
                nc.sync.dma_start(out=sin_m, in_=sin2[m * 128:(m + 1) * 128, :])

                # ---- qkv: psq[s128, O] = x[s128, :] @ Wc.T ----
                psq = psA.tile([128, O], F32, tag="A")
                for k in range(16):
                    nc.tensor.matmul(
                        psq,
                        lhsT=xTm[:, k, :],
                        rhs=wct_sb[:, k, :],
                        start=(k == 0),
                        stop=(k == 15),
                    )

                # ---- rmsnorm stats over q,k (5 groups of 64) ----
                qk = psq[:, 0:QO + HD]
                qk5 = qk.rearrange("p (h d) -> p h d", d=HD)
                sq = workp.tile([128, QO + HD], F32, tag="sq")
                nc.scalar.square(sq, qk)
                ssum = smallp.tile([128, 5], F32, tag="ssum")
                nc.vector.tensor_reduce(
                    ssum,
                    sq.rearrange("p (h d) -> p h d", d=HD),
                    axis=mybir.AxisListType.X,
                    op=ALU.add,
                )
                rstd = smallp.tile([128, 5], F32, tag="rstd")
                nc.scalar.activation(
                    rstd, ssum, AF.Sqrt, bias=eps_sb, scale=1.0 / HD
                )
                nc.vector.reciprocal(rstd, rstd)

                # ---- rope: qkr = qk*cos + rot_half(qk)*sin  (then *rstd) ----
                rot = workp.tile([128, QO + HD], F32, tag="rot")
                rot5 = rot.rearrange("p (h d) -> p h d", d=HD)
                nc.vector.tensor_scalar_mul(rot5[:, :, 0:32], qk5[:, :, 32:64], -1.0)
                nc.vector.tensor_scalar_mul(rot5[:, :, 32:64], qk5[:, :, 0:32], 1.0)

                cos_q4 = cos_m[:, 0:HD].unsqueeze(1).broadcast_to([128, HPC, HD])
                sin_q4 = sin_m[:, 0:HD].unsqueeze(1).broadcast_to([128, HPC, HD])

                qkr = workp.tile([128, QO + HD], F32, tag="qkr")
                qkr5 = qkr.rearrange("p (h d) -> p h d", d=HD)
                nc.vector.tensor_tensor(
                    qkr5[:, 0:HPC, :], qk5[:, 0:HPC, :], cos_q4, ALU.mult
                )
                nc.vector.tensor_tensor(
                    qkr[:, QO:QO + HD], qk[:, QO:QO + HD], cos_m[:, HD:2 * HD],
                    ALU.mult,
                )
                nc.gpsimd.tensor_tensor(
                    rot5[:, 0:HPC, :], rot5[:, 0:HPC, :], sin_q4, ALU.mult
                )
                nc.gpsimd.tensor_tensor(
                    rot[:, QO:QO + HD], rot[:, QO:QO + HD], sin_m[:, HD:2 * HD],
                    ALU.mult,
                )
                nc.vector.tensor_add(qkr, qkr, rot)
                for h in range(5):
                    nc.vector.tensor_scalar_mul(
                        qkr[:, h * HD:(h + 1) * HD],
                        qkr[:, h * HD:(h + 1) * HD],
                        rstd[:, h:h + 1],
                    )

                # ---- v (bf16, with trailing ones column for softmax denom) ----
                nc.scalar.copy(v_sb[:, m, 0:HD], psq[:, QO + HD:O])

                # ---- transposes into [dim, s] layout ----
                for h in range(HPC):
                    trh = psTR.tile([128, 128], F32, tag="tr")
                    nc.tensor.transpose(
                        trh[0:64, :], qkr[:, h * HD:(h + 1) * HD], ident
                    )
                    nc.scalar.copy(qT[h][:, m * 128:(m + 1) * 128], trh[0:64, :])
                trk = psTR.tile([128, 128], F32, tag="tr")
                nc.tensor.transpose(trk[0:64, :], qkr[:, QO:QO + HD], ident)
                nc.scalar.copy(kT[:, m * 128:(m + 1) * 128], trk[0:64, :])

            if m >= 4:
                kb = m - 4
                w = min(WMAX, S - kb * 128)
                wA = min(w, 384)
                wB = w - wA
                lhsk = kT[:, kb * 128:(kb + 1) * 128]

                # ---- scores (transposed: [k, q]) + exp + masks ----
                for h in range(HPC):
                    rq = qT[h][:, kb * 128:kb * 128 + w]
                    probs = probsp.tile([128, WMAX], BF16, tag="probs")
                    scA = psSC.tile([128, 384], F32, tag="scA")
                    nc.tensor.matmul(
                        scA[:, 0:wA],
                        lhsT=lhsk,
                        rhs=rq[:, 0:wA],
                        start=True,
                        stop=True,
                    )
                    nc.scalar.activation(probs[:, 0:wA], scA[:, 0:wA], AF.Exp)
                    if wB > 0:
                        scB = psSC.tile([128, 256], F32, tag="scB")
                        nc.tensor.matmul(
                            scB[:, 0:wB],
                            lhsT=lhsk,
                            rhs=rq[:, wA:w],
                            start=True,
                            stop=True,
                        )
                        nc.scalar.activation(probs[:, wA:w], scB[:, 0:wB], AF.Exp)
                    # causal mask on the diagonal block: keep iff q >= k
                    nc.gpsimd.affine_select(
                        out=probs[:, 0:128],
                        in_=probs[:, 0:128],
                        pattern=[[1, 128]],
                        compare_op=ALU.is_ge,
                        fill=0.0,
                        base=0,
                        channel_multiplier=-1,
                    )
                    # sliding-window mask on the last block: keep iff q-k < 512
                    if w == WMAX:
                        nc.gpsimd.affine_select(
                            out=probs[:, 512:640],
                            in_=probs[:, 512:640],
                            pattern=[[-1, 128]],
                            compare_op=ALU.is_gt,
                            fill=0.0,
                            base=0,
                            channel_multiplier=1,
                        )
                    probs_tiles[(kb, h)] = probs

                # ---- PV + normalize for query block qb = kb ----
                qb = kb
                kbs = list(range(max(0, qb - 4), qb + 1))
                attnA = attnp.tile([128, 128], F32, tag="attnA")
                attnB = attnp.tile([128, 128], F32, tag="attnB")
                for h in range(HPC):
                    pv = psPV.tile([128, HD + 1], F32, tag="pv")
                    for i, k2 in enumerate(kbs):
                        pt = probs_tiles[(k2, h)]
                        off = (qb - k2) * 128
                        nc.tensor.matmul(
                            pv,
                            lhsT=pt[:, off:off + 128],
                            rhs=v_sb[:, k2, :],
                            start=(i == 0),
                            stop=(i == len(kbs) - 1),
                        )
                    rec = smallp.tile([128, 1], F32, tag="rec")
                    nc.vector.reciprocal(rec, pv[:, HD:HD + 1])
                    att = attnA if h < 2 else attnB
                    nc.scalar.activation(
                        att[:, (h % 2) * HD:(h % 2) * HD + HD],
                        pv[:, 0:HD],
                        AF.Copy,
                        scale=rec,
                    )

                # ---- transpose attn to [o, s], partial out-proj ----
                trA = psTR.tile([128, 128], F32, tag="tr")
                nc.tensor.transpose(trA, attnA, ident)
                aT1 = attnp.tile([128, 128], F32R, tag="aT1")
                nc.scalar.copy(aT1, trA)
                trB = psTR.tile([128, 128], F32, tag="tr")
                nc.tensor.transpose(trB, attnB, ident)
                aT2 = attnp.tile([128, 128], F32R, tag="aT2")
                nc.scalar.copy(aT2, trB)

                outsb = outsp.tile([128, D_MODEL], F32, tag="outsb")
                for n in range(4):
                    po = psA.tile([128, 512], F32, tag="A")
                    nc.tensor.matmul(
                        po,
                        lhsT=aT1,
                        rhs=wot_sb[:, 0, n * 512:(n + 1) * 512],
                        start=True,
                        stop=False,
                    )
                    nc.tensor.matmul(
                        po,
                        lhsT=aT2,
                        rhs=wot_sb[:, 1, n * 512:(n + 1) * 512],
                        start=False,
                        stop=True,
                    )
                    nc.vector.tensor_copy(outsb[:, n * 512:(n + 1) * 512], po)
                nc.sync.dma_start(out=out[qb * 128:(qb + 1) * 128, :], in_=outsb)


def _get_nc():
    global _NC
    if _NC is None:
        nc = bacc.Bacc(trn_type="TRN2")
        xT = nc.dram_tensor("xT", [D_MODEL, S], F32R, kind="ExternalInput").ap()
        wct = nc.dram_tensor("wct", [D_MODEL, O], F32R, kind="ExternalInput").ap()
        wot = nc.dram_tensor("wot", [QO, D_MODEL], F32R, kind="ExternalInput").ap()
        cos2 = nc.dram_tensor("cos2", [S, 2 * HD], F32, kind="ExternalInput").ap()
        sin2 = nc.dram_tensor("sin2", [S, 2 * HD], F32, kind="ExternalInput").ap()
        out = nc.dram_tensor("out", [S, D_MODEL], F32, kind="ExternalOutput").ap()
        with tile.TileContext(nc) as tc:
            _body(tc, xT, wct, wot, cos2, sin2, out)
        nc.compile()
        _NC = nc
    return _NC


LAST_RESULTS = None


def kernel(x, w_qkv, w_out, q_norm_w, k_norm_w):
    global LAST_RESULTS
    x = np.asarray(x, np.float32)
    w_qkv = np.asarray(w_qkv, np.float32)
    w_out = np.asarray(w_out, np.float32)
    qw = np.asarray(q_norm_w, np.float32)
    kw = np.asarray(k_norm_w, np.float32)

    xT = np.ascontiguousarray(x[0].T)  # [D, S]

    inv_freq = (1.0 / (THETA ** (np.arange(0, HD, 2, dtype=np.float32) / HD))).astype(
        np.float32
    )
    pos = np.arange(S, dtype=np.float32)
    ang = pos[:, None] * inv_freq[None, :]
    emb = np.concatenate([ang, ang], axis=1).astype(np.float32)
    cos = np.cos(emb).astype(np.float32)
    sin = np.sin(emb).astype(np.float32)
    qw_rot = np.concatenate([qw[32:], qw[:32]])
    kw_rot = np.concatenate([kw[32:], kw[:32]])
    cos2 = np.ascontiguousarray(
        np.concatenate([cos * (qw * SCALE)[None, :], cos * kw[None, :]], axis=1)
    )
    sin2 = np.ascontiguousarray(
        np.concatenate(
            [sin * (qw_rot * SCALE)[None, :], sin * kw_rot[None, :]], axis=1
        )
    )

    q_size = N_HEADS * HD
    kv_size = N_KV_HEADS * HD
    wq = w_qkv[0:q_size].reshape(N_HEADS, HD, D_MODEL)
    wk = w_qkv[q_size:q_size + kv_size].reshape(N_KV_HEADS, HD, D_MODEL)
    wv = w_qkv[q_size + kv_size:].reshape(N_KV_HEADS, HD, D_MODEL)

    in_maps = []
    for c in range(NCORES):
        Wc = np.concatenate(
            [wq[c * HPC:(c + 1) * HPC].reshape(QO, D_MODEL), wk[c], wv[c]], axis=0
        )
        in_maps.append(
            {
                "xT": xT,
                "wct": np.ascontiguousarray(Wc.T),
                "wot": np.ascontiguousarray(w_out[:, c * QO:(c + 1) * QO].T),
                "cos2": cos2,
                "sin2": sin2,
            }
        )

    nc = _get_nc()
    res = run_bass_kernel_spmd(
        nc,
        in_maps,
        list(range(NCORES)),
        trace=bool(os.environ.get("KERNEL_TRACE")),
    )
    LAST_RESULTS = res
    acc = res.results[0]["out"].astype(np.float32).copy()
    for c in range(1, NCORES):
        acc += res.results[c]["out"]
    return acc.reshape(1, S, D_MODEL)


# revision 17
# speedup vs baseline: 1.1012x; 1.1012x over previous
"""GQA sliding-window attention block, tensor-parallel over 8 TRN2 NeuronCores.

Each core owns 4 query heads + their shared KV head (GQA group), computes
qkv-proj for its slice, banded flash-style attention (sliding window 512
=> only 5 key blocks of 128 per 128-query block), and a partial
out-projection (contraction over its 256 attention-output dims).  The host
sums the 8 partial outputs.

All matmuls run in bf16 (fp32 PSUM accumulate); softmax statistics and
normalisation stay fp32.  Measured end-to-end relative error ~5e-3.

Per-core pipeline is striped over the 16 blocks of 128 sequence positions:
  stripe m: qkv matmul -> rmsnorm+rope (fused tables) -> PE transposes into
            qT/kT -> scores for key-block m-4 (needs q up to block m) ->
            mask (DVE, on psum) -> exp (ACT, psum->sbuf bf16) -> PV for
            query-block m-4 -> out-proj -> DMA partial out.
"""

import os
import sys

import numpy as np

if "/opt/trn_rl_repo" not in sys.path:
    sys.path.insert(0, "/opt/trn_rl_repo")

import ml_dtypes

import concourse.bass as bass
import concourse.mybir as mybir
import concourse.tile as tile
from concourse import bacc
from concourse.bass_utils import run_bass_kernel_spmd
from concourse.masks import make_identity

F32 = mybir.dt.float32
BF16 = mybir.dt.bfloat16
AF = mybir.ActivationFunctionType
ALU = mybir.AluOpType

N_HEADS = 32
N_KV_HEADS = 8
HD = 64
D_MODEL = 2048
S = 2048
WINDOW = 512
THETA = 10000.0
EPS = 1e-6
NCORES = 8
HPC = N_HEADS // NCORES          # 4 q heads per core
QO = HPC * HD                    # 256 q cols per core
O = QO + 2 * HD                  # 384 qkv cols per core
SCALE = HD ** -0.5
NS = S // 128                    # 16 stripes
KBW = WINDOW // 128 + 1          # 5 key blocks per query block
WMAX = KBW * 128                 # 640

_NC = None


def _body(tc, xT, wct, wot, cos2, sin2, out):
    nc = tc.nc
    with (
        tc.tile_pool(name="const", bufs=1) as constp,
        tc.tile_pool(name="xk", bufs=2) as xkp,
        tc.tile_pool(name="work", bufs=2) as workp,
        tc.tile_pool(name="small", bufs=4) as smallp,
        tc.tile_pool(name="persist", bufs=1) as persistp,
        tc.tile_pool(name="probs", bufs=24) as probsp,
        tc.tile_pool(name="attn", bufs=2) as attnp,
        tc.tile_pool(name="outs", bufs=2) as outsp,
        tc.tile_pool(name="psA", bufs=2, space="PSUM") as psA,
        tc.tile_pool(name="psSC", bufs=2, space="PSUM") as psSC,
        tc.tile_pool(name="psPV", bufs=2, space="PSUM") as psPV,
    ):
        ident_f = constp.tile([128, 128], F32)
        make_identity(nc, ident_f)
        eps_sb = constp.tile([128, 1], F32)
        nc.vector.memset(eps_sb, EPS)
        # binary masks (bf16): dmask keeps q>=k on the diagonal block,
        # wmask keeps q-k<512 on the trailing window block
        dmask = constp.tile([128, 128], BF16)
        nc.vector.memset(dmask, 1.0)
        nc.gpsimd.affine_select(
            out=dmask, in_=dmask, pattern=[[1, 128]], compare_op=ALU.is_ge,
            fill=0.0, base=0, channel_multiplier=-1,
        )
        wmask = constp.tile([128, 128], BF16)
        nc.vector.memset(wmask, 1.0)
        nc.gpsimd.affine_select(
            out=wmask, in_=wmask, pattern=[[-1, 128]], compare_op=ALU.is_gt,
            fill=0.0, base=0, channel_multiplier=1,
        )

        # x resident in SBUF: [p = d%128, kc = d//128, s], bf16
        x_all = constp.tile([128, 16, S], BF16)
        nc.sync.dma_start(
            out=x_all, in_=xT.rearrange("(kc p) s -> p kc s", p=128)
        )
        wct_sb = constp.tile([128, 16, O], BF16)
        nc.sync.dma_start(out=wct_sb, in_=wct.rearrange("(kc p) o -> p kc o", p=128))
        wot_sb = constp.tile([128, 2, D_MODEL], BF16)
        nc.sync.dma_start(out=wot_sb, in_=wot.rearrange("(oc p) n -> p oc n", p=128))

        qT = [
            persistp.tile([64, S], BF16, name=f"qT{h}", tag=f"qT{h}")
            for h in range(HPC)
        ]
        kT = persistp.tile([64, S], BF16)
        v_sb = persistp.tile([128, NS, HD + 1], BF16)
        nc.vector.memset(v_sb[:, :, HD:HD + 1], 1.0)

        probs_tiles = {}

        for m in range(NS + 4):
            if m < NS:
                cos_m = xkp.tile([128, 2 * HD], F32, tag="cosm")
                nc.sync.dma_start(out=cos_m, in_=cos2[m * 128:(m + 1) * 128, :])
                sin_m = xkp.tile([128, 2 * HD], F32, tag="sinm")
                nc.sync.dma_start(out=sin_m, in_=sin2[m * 128:(m + 1) * 128, :])

                # ---- qkv: psq[s128, O] = x[s128, :] @ Wc.T  (bf16) ----
                psq = psA.tile([128, O], F32, tag="A")
                for k in range(16):
                    nc.tensor.matmul(
                        psq,
                        lhsT=x_all[:, k, m * 128:(m + 1) * 128],
                        rhs=wct_sb[:, k, :],
                        start=(k == 0),
                        stop=(k == 15),
                    )

                # ---- rmsnorm stats over q,k (5 groups of 64) ----
                qk = psq[:, 0:QO + HD]
                qk5 = qk.rearrange("p (h d) -> p h d", d=HD)
                sq = workp.tile([128, QO + HD], F32, tag="sq")
                nc.scalar.square(sq, qk)
                ssum = smallp.tile([128, 5], F32, tag="ssum")
                nc.vector.tensor_reduce(
                    ssum,
                    sq.rearrange("p (h d) -> p h d", d=HD),
                    axis=mybir.AxisListType.X,
                    op=ALU.add,
                )
                rstd = smallp.tile([128, 5], F32, tag="rstd")
                nc.scalar.activation(
                    rstd, ssum, AF.Sqrt, bias=eps_sb, scale=1.0 / HD
                )
                nc.vector.reciprocal(rstd, rstd)

                # ---- rope: qkr = qk*cos + rot_half(qk)*sin, then *rstd -> bf16
                rot = workp.tile([128, QO + HD], F32, tag="rot")
                rot5 = rot.rearrange("p (h d) -> p h d", d=HD)
                nc.vector.tensor_scalar_mul(rot5[:, :, 0:32], qk5[:, :, 32:64], -1.0)
                nc.vector.tensor_scalar_mul(rot5[:, :, 32:64], qk5[:, :, 0:32], 1.0)

                cos_q4 = cos_m[:, 0:HD].unsqueeze(1).broadcast_to([128, HPC, HD])
                sin_q4 = sin_m[:, 0:HD].unsqueeze(1).broadcast_to([128, HPC, HD])

                qkr = workp.tile([128, QO + HD], F32, tag="qkr")
                qkr5 = qkr.rearrange("p (h d) -> p h d", d=HD)
                nc.vector.tensor_tensor(
                    qkr5[:, 0:HPC, :], qk5[:, 0:HPC, :], cos_q4, ALU.mult
                )
                nc.vector.tensor_tensor(
                    qkr[:, QO:QO + HD], qk[:, QO:QO + HD], cos_m[:, HD:2 * HD],
                    ALU.mult,
                )
                nc.gpsimd.tensor_tensor(
                    rot5[:, 0:HPC, :], rot5[:, 0:HPC, :], sin_q4, ALU.mult
                )
                nc.gpsimd.tensor_tensor(
                    rot[:, QO:QO + HD], rot[:, QO:QO + HD], sin_m[:, HD:2 * HD],
                    ALU.mult,
                )
                nc.vector.tensor_add(qkr, qkr, rot)
                for h in range(5):
                    nc.vector.tensor_scalar_mul(
                        qkr[:, h * HD:(h + 1) * HD],
                        qkr[:, h * HD:(h + 1) * HD],
                        rstd[:, h:h + 1],
                    )

                # ---- v (bf16, with trailing ones column for softmax denom) ----
                nc.scalar.copy(v_sb[:, m, 0:HD], psq[:, QO + HD:O])

                # ---- transposes into [dim, s] layout (bf16 in, fp32 psum) ----
                for h in range(HPC):
                    trh = psSC.tile([128, 128], F32, tag="sm")
                    nc.tensor.transpose(
                        trh[0:64, :], qkr[:, h * HD:(h + 1) * HD], ident_f
                    )
                    nc.scalar.copy(qT[h][:, m * 128:(m + 1) * 128], trh[0:64, :])
                trk = psSC.tile([128, 128], F32, tag="sm")
                nc.tensor.transpose(trk[0:64, :], qkr[:, QO:QO + HD], ident_f)
                nc.scalar.copy(kT[:, m * 128:(m + 1) * 128], trk[0:64, :])

            if m >= 4:
                kb = m - 4
                w = min(WMAX, S - kb * 128)
                wA = min(w, 512)
                wB = w - wA          # 128 for interior kb, 0 at the tail
                lhsk = kT[:, kb * 128:(kb + 1) * 128]

                # ---- scores (transposed: [k, q]), mask on psum, exp ----
                for h in range(HPC):
                    rq = qT[h][:, kb * 128:kb * 128 + w]
                    probs = probsp.tile([128, WMAX], BF16, tag="probs")
                    scA = psSC.tile([128, 512], F32, tag="scA")
                    nc.tensor.matmul(
                        scA[:, 0:wA],
                        lhsT=lhsk,
                        rhs=rq[:, 0:wA],
                        start=True,
                        stop=True,
                    )
                    nc.scalar.activation(probs[:, 0:wA], scA[:, 0:wA], AF.Exp)
                    nc.vector.tensor_tensor(
                        probs[:, 0:128], probs[:, 0:128], dmask, ALU.mult
                    )
                    if wB > 0:
                        scB = psSC.tile([128, 128], F32, tag="sm")
                        nc.tensor.matmul(
                            scB[:, 0:wB],
                            lhsT=lhsk,
                            rhs=rq[:, wA:w],
                            start=True,
                            stop=True,
                        )
                        nc.scalar.activation(probs[:, wA:w], scB[:, 0:wB], AF.Exp)
                        nc.vector.tensor_tensor(
                            probs[:, wA:w], probs[:, wA:w], wmask, ALU.mult
                        )
                    probs_tiles[(kb, h)] = probs

                # ---- PV for query block qb = kb (4 heads share one bank) ----
                qb = kb
                kbs = list(range(max(0, qb - 4), qb + 1))
                pv4 = psPV.tile([128, HPC, 72], F32, tag="pv")
                for i, k2 in enumerate(kbs):
                    off = (qb - k2) * 128
                    for h in range(HPC):
                        nc.tensor.matmul(
                            pv4[:, h, 0:HD + 1],
                            lhsT=probs_tiles[(k2, h)][:, off:off + 128],
                            rhs=v_sb[:, k2, :],
                            start=(i == 0 and h == 0),
                            stop=(i == len(kbs) - 1 and h == HPC - 1),
                        )
                attnA = attnp.tile([128, 128], F32, tag="attnA")
                attnB = attnp.tile([128, 128], F32, tag="attnB")
                for h in range(HPC):
                    rec = smallp.tile([128, 1], F32, tag="rec")
                    nc.vector.reciprocal(rec, pv4[:, h, HD:HD + 1])
                    att = attnA if h < 2 else attnB
                    nc.scalar.activation(
                        att[:, (h % 2) * HD:(h % 2) * HD + HD],
                        pv4[:, h, 0:HD],
                        AF.Copy,
                        scale=rec,
                    )

                # ---- transpose attn to [o, s], partial out-proj (bf16) ----
                trA = psSC.tile([128, 128], F32, tag="sm")
                nc.tensor.transpose(trA, attnA, ident_f)
                aT1 = attnp.tile([128, 128], BF16, tag="aT1")
                nc.scalar.copy(aT1, trA)
                trB = psSC.tile([128, 128], F32, tag="sm")
                nc.tensor.transpose(trB, attnB, ident_f)
                aT2 = attnp.tile([128, 128], BF16, tag="aT2")
                nc.scalar.copy(aT2, trB)

                outsb = outsp.tile([128, D_MODEL], F32, tag="outsb")
                for n in range(4):
                    po = psA.tile([128, 512], F32, tag="A")
                    nc.tensor.matmul(
                        po,
                        lhsT=aT1,
                        rhs=wot_sb[:, 0, n * 512:(n + 1) * 512],
                        start=True,
                        stop=False,
                    )
                    nc.tensor.matmul(
                        po,
                        lhsT=aT2,
                        rhs=wot_sb[:, 1, n * 512:(n + 1) * 512],
                        start=False,
                        stop=True,
                    )
                    if n % 2 == 0:
                        nc.vector.tensor_copy(outsb[:, n * 512:(n + 1) * 512], po)
                    else:
                        nc.scalar.copy(outsb[:, n * 512:(n + 1) * 512], po)
                nc.sync.dma_start(out=out[qb * 128:(qb + 1) * 128, :], in_=outsb)


def _get_nc():
    global _NC
    if _NC is None:
        nc = bacc.Bacc(trn_type="TRN2")
        xT = nc.dram_tensor("xT", [D_MODEL, S], BF16, kind="ExternalInput").ap()
        wct = nc.dram_tensor("wct", [D_MODEL, O], BF16, kind="ExternalInput").ap()
        wot = nc.dram_tensor("wot", [QO, D_MODEL], BF16, kind="ExternalInput").ap()
        cos2 = nc.dram_tensor("cos2", [S, 2 * HD], F32, kind="ExternalInput").ap()
        sin2 = nc.dram_tensor("sin2", [S, 2 * HD], F32, kind="ExternalInput").ap()
        out = nc.dram_tensor("out", [S, D_MODEL], F32, kind="ExternalOutput").ap()
        with tile.TileContext(nc) as tc:
            _body(tc, xT, wct, wot, cos2, sin2, out)
        nc.compile()
        _NC = nc
    return _NC


LAST_RESULTS = None


def kernel(x, w_qkv, w_out, q_norm_w, k_norm_w):
    global LAST_RESULTS
    x = np.asarray(x, np.float32)
    w_qkv = np.asarray(w_qkv, np.float32)
    w_out = np.asarray(w_out, np.float32)
    qw = np.asarray(q_norm_w, np.float32)
    kw = np.asarray(k_norm_w, np.float32)
    bf = ml_dtypes.bfloat16

    xT = np.ascontiguousarray(x[0].T).astype(bf)  # [D, S]

    inv_freq = (1.0 / (THETA ** (np.arange(0, HD, 2, dtype=np.float32) / HD))).astype(
        np.float32
    )
    pos = np.arange(S, dtype=np.float32)
    ang = pos[:, None] * inv_freq[None, :]
    emb = np.concatenate([ang, ang], axis=1).astype(np.float32)
    cos = np.cos(emb).astype(np.float32)
    sin = np.sin(emb).astype(np.float32)
    qw_rot = np.concatenate([qw[32:], qw[:32]])
    kw_rot = np.concatenate([kw[32:], kw[:32]])
    cos2 = np.ascontiguousarray(
        np.concatenate([cos * (qw * SCALE)[None, :], cos * kw[None, :]], axis=1)
    )
    sin2 = np.ascontiguousarray(
        np.concatenate(
            [sin * (qw_rot * SCALE)[None, :], sin * kw_rot[None, :]], axis=1
        )
    )

    q_size = N_HEADS * HD
    kv_size = N_KV_HEADS * HD
    wq = w_qkv[0:q_size].reshape(N_HEADS, HD, D_MODEL)
    wk = w_qkv[q_size:q_size + kv_size].reshape(N_KV_HEADS, HD, D_MODEL)
    wv = w_qkv[q_size + kv_size:].reshape(N_KV_HEADS, HD, D_MODEL)

    in_maps = []
    for c in range(NCORES):
        Wc = np.concatenate(
            [wq[c * HPC:(c + 1) * HPC].reshape(QO, D_MODEL), wk[c], wv[c]], axis=0
        )
        in_maps.append(
            {
                "xT": xT,
                "wct": np.ascontiguousarray(Wc.T).astype(bf),
                "wot": np.ascontiguousarray(
                    w_out[:, c * QO:(c + 1) * QO].T
                ).astype(bf),
                "cos2": cos2,
                "sin2": sin2,
            }
        )

    nc = _get_nc()
    res = run_bass_kernel_spmd(
        nc,
        in_maps,
        list(range(NCORES)),
        trace=bool(os.environ.get("KERNEL_TRACE")),
    )
    LAST_RESULTS = res
    acc = res.results[0]["out"].astype(np.float32).copy()
    for c in range(1, NCORES):
        acc += res.results[c]["out"]
    return acc.reshape(1, S, D_MODEL)
